# revision 6
# baseline (speedup 1.0000x reference)
"""Sliding-window attention (B=2, S=2048, D=2048, H=16, HD=128, W=256) on 8
Trainium2 NeuronCores.

Sharding: data-parallel on batch (2) x sequence-parallel (4 chunks of 512
queries). Each core recomputes the K/V projections for its 256-position halo,
so there are no collectives; the host gathers the 8 output slices.

Per-core pipeline (all matmuls bf16 with f32 PSUM accumulation):
  1. V = x @ wv.T      (x-stationary, output in [seq, feat] layout)
  2. K,Q = x @ w.T     (weight-stationary, output transposed [feat, seq]),
     RoPE applied via a half-swap permutation matmul + two DVE multiplies.
     Weight rows are host-permuted so each head's features are
     [re(0..63), im(0..63)], making RoPE a half-tile rotate.
  3. Banded attention: per (head, 128-query block) only the 3 key blocks
     covering the 256-wide window are computed. Scores are built transposed
     ([key, query]) so softmax normalization works via ones-matmul column
     sums; masking is multiplicative {0,1} after exp (no max subtraction —
     scores are bounded by construction).
  4. out = att @ wo.T  (weight-stationary, output transposed; host untransposes).
"""

import math

import numpy as np
import ml_dtypes

B, S, D = 2, 2048, 2048
H = 16
HD = 128
W = 256
NCORES = 8
SC = 512            # query positions per core
KV = SC + W         # 768 key/value positions per core
NDB = D // 128      # 16 contraction blocks
SCALE = 1.0 / math.sqrt(HD)

bf16 = ml_dtypes.bfloat16

_CACHE = {}


def _build_program():
    import concourse.bass as bass
    import concourse.mybir as mybir
    import concourse.tile as tile

    BF16 = mybir.dt.bfloat16
    FP32 = mybir.dt.float32
    Exp = mybir.ActivationFunctionType.Exp

    nc = bass.Bass()

    xT = nc.declare_dram_parameter("xT", [128, NDB, KV], BF16, isOutput=False)
    wqt = nc.declare_dram_parameter("wqt", [H, 128, NDB, 128], BF16, isOutput=False)
    wkt = nc.declare_dram_parameter("wkt", [H, 128, NDB, 128], BF16, isOutput=False)
    wvt = nc.declare_dram_parameter("wvt", [4, 128, NDB, 512], BF16, isOutput=False)
    wot = nc.declare_dram_parameter("wot", [16, 128, NDB, 128], BF16, isOutput=False)
    tabc = nc.declare_dram_parameter("tabc", [128, KV], BF16, isOutput=False)
    tabsn = nc.declare_dram_parameter("tabsn", [128, KV], BF16, isOutput=False)
    masks = nc.declare_dram_parameter("masks", [128, 12, 128], BF16, isOutput=False)
    p128 = nc.declare_dram_parameter("p128", [128, 128], BF16, isOutput=False)
    outT = nc.declare_dram_parameter("outT", [D, SC], FP32, isOutput=True)

    with tile.TileContext(nc) as tc:
        with tc.tile_pool(name="const", bufs=1) as singles, \
             tc.tile_pool(name="wts", bufs=1) as wpool, \
             tc.tile_pool(name="rope", bufs=1) as rpool, \
             tc.tile_pool(name="att", bufs=1) as apool, \
             tc.tile_pool(name="outp", bufs=1) as opool:

            # --- resident inputs / constants ---
            xT_sb = singles.tile([128, NDB, KV], BF16)
            nc.sync.dma_start(out=xT_sb, in_=xT[:, :, :])
            tabc_sb = singles.tile([128, KV], BF16)
            nc.sync.dma_start(out=tabc_sb, in_=tabc[:, :])
            tabs_sb = singles.tile([128, KV], BF16)
            nc.sync.dma_start(out=tabs_sb, in_=tabsn[:, :])
            masks_sb = singles.tile([128, 12, 128], BF16)
            nc.sync.dma_start(out=masks_sb, in_=masks[:, :, :])
            p_sb = singles.tile([128, 128], BF16)
            nc.sync.dma_start(out=p_sb, in_=p128[:, :])
            ones_col = singles.tile([128, 1], BF16)
            nc.vector.memset(ones_col, 1.0)
            ones_row = singles.tile([1, 128], FP32)
            nc.vector.memset(ones_row, 1.0)

            # --- resident intermediates ---
            k_sb = singles.tile([128, H, KV], BF16)      # [hd, h, key pos]
            q_sb = singles.tile([128, H, SC], BF16)      # [hd, h, query pos]
            v_sb = singles.tile([128, KV // 128, D], BF16)  # [pos%128, pos//128, feat]
            att_sb = singles.tile([128, H, SC], BF16)    # [hd, h, query pos]

            # ---------------- phase 1: projections ----------------
            with tc.tile_pool(name="pp1", bufs=1, space="PSUM") as pp1:
                # V projection: x-stationary, normal [seq, feat] output
                for oc in range(4):
                    wv_t = wpool.tile([128, NDB, 512], BF16, tag="wv", bufs=2)
                    nc.sync.dma_start(out=wv_t, in_=wvt[oc])
                    for rb in range(KV // 128):
                        ps = pp1.tile([128, 512], FP32, tag="big", bufs=3)
                        for db in range(NDB):
                            nc.tensor.matmul(
                                ps,
                                lhsT=xT_sb[:, db, rb * 128:(rb + 1) * 128],
                                rhs=wv_t[:, db, :],
                                start=(db == 0),
                                stop=(db == NDB - 1),
                            )
                        nc.scalar.copy(
                            out=v_sb[:, rb, oc * 512:(oc + 1) * 512], in_=ps
                        )

                # K and Q projections: weight-stationary, transposed output
                def proj_rope(w_dram, dst, dst_off, r0, rn):
                    # dst[:, h, dst_off:dst_off+rn] = RoPE(w.T @ x[:, r0:r0+rn])
                    for h in range(H):
                        w_t = wpool.tile([128, NDB, 128], BF16, tag="wqk", bufs=3)
                        nc.sync.dma_start(out=w_t, in_=w_dram[h])
                        for c0 in range(0, rn, 512):
                            cn = min(512, rn - c0)
                            a0 = r0 + c0          # column offset into xT / tabs
                            ps = pp1.tile([128, 512], FP32, tag="big", bufs=3)
                            for db in range(NDB):
                                nc.tensor.matmul(
                                    ps[:, :cn],
                                    lhsT=w_t[:, db, :],
                                    rhs=xT_sb[:, db, a0:a0 + cn],
                                    start=(db == 0),
                                    stop=(db == NDB - 1),
                                )
                            raw = rpool.tile([128, 512], BF16, tag="raw", bufs=3)
                            nc.scalar.copy(out=raw[:, :cn], in_=ps[:, :cn])
                            psw = pp1.tile([128, 512], FP32, tag="swap", bufs=2)
                            nc.tensor.matmul(
                                psw[:, :cn], lhsT=p_sb, rhs=raw[:, :cn],
                                start=True, stop=True,
                            )
                            tc_ = rpool.tile([128, 512], BF16, tag="tc", bufs=3)
                            nc.vector.tensor_mul(
                                tc_[:, :cn], raw[:, :cn], tabc_sb[:, a0:a0 + cn]
                            )
                            ts_ = rpool.tile([128, 512], BF16, tag="ts", bufs=3)
                            nc.vector.tensor_mul(
                                ts_[:, :cn], psw[:, :cn], tabs_sb[:, a0:a0 + cn]
                            )
                            o0 = dst_off + c0
                            nc.vector.tensor_add(
                                dst[:, h, o0:o0 + cn], tc_[:, :cn], ts_[:, :cn]
                            )

                proj_rope(wkt, k_sb, 0, 0, KV)
                proj_rope(wqt, q_sb, 0, W, SC)

            # ---------------- phase 2: banded attention ----------------
            with tc.tile_pool(name="pp2", bufs=1, space="PSUM") as pp2:
                for h in range(H):
                    for t in range(4):
                        ps_s = pp2.tile([128, 3, 128], FP32, tag="s", bufs=2)
                        for blk in range(3):
                            kb = t + blk
                            nc.tensor.matmul(
                                ps_s[:, blk, :],
                                lhsT=k_sb[:, h, kb * 128:(kb + 1) * 128],
                                rhs=q_sb[:, h, t * 128:(t + 1) * 128],
                                start=True,
                                stop=True,
                            )
                        em = apool.tile([128, 3, 128], BF16, tag="em", bufs=3)
                        e = apool.tile([128, 3, 128], BF16, tag="e", bufs=2)
                        for blk in range(3):
                            nc.scalar.activation(
                                e[:, blk, :], ps_s[:, blk, :], Exp, scale=SCALE
                            )
                            nc.vector.tensor_mul(
                                em[:, blk, :], e[:, blk, :],
                                masks_sb[:, t * 3 + blk, :],
                            )
                        ps_d = pp2.tile([1, 128], FP32, tag="d", bufs=2)
                        for blk in range(3):
                            nc.tensor.matmul(
                                ps_d,
                                lhsT=ones_col,
                                rhs=em[:, blk, :],
                                start=(blk == 0),
                                stop=(blk == 2),
                            )
                        rinv = apool.tile([1, 128], FP32, tag="rinv", bufs=2)
                        nc.vector.reciprocal(rinv, ps_d)
                        ps_bc = pp2.tile([128, 128], FP32, tag="bc", bufs=2)
                        nc.tensor.matmul(
                            ps_bc, lhsT=ones_row, rhs=rinv, start=True, stop=True
                        )
                        rbc = apool.tile([128, 128], FP32, tag="rbc", bufs=2)
                        nc.scalar.copy(out=rbc, in_=ps_bc)
                        ps_pv = pp2.tile([128, 128], FP32, tag="pv", bufs=2)
                        for blk in range(3):
                            nc.tensor.matmul(
                                ps_pv,
                                lhsT=v_sb[:, t + blk, h * 128:(h + 1) * 128],
                                rhs=em[:, blk, :],
                                start=(blk == 0),
                                stop=(blk == 2),
                            )
                        nc.vector.tensor_mul(
                            att_sb[:, h, t * 128:(t + 1) * 128], ps_pv, rbc
                        )

            # ---------------- phase 3: output projection ----------------
            with tc.tile_pool(name="pp3", bufs=1, space="PSUM") as pp3:
                for ob in range(16):
                    wo_t = wpool.tile([128, NDB, 128], BF16, tag="wqk", bufs=3)
                    nc.sync.dma_start(out=wo_t, in_=wot[ob])
                    ps_o = pp3.tile([128, 512], FP32, tag="wo", bufs=3)
                    for fb in range(H):
                        nc.tensor.matmul(
                            ps_o,
                            lhsT=wo_t[:, fb, :],
                            rhs=att_sb[:, fb, :],
                            start=(fb == 0),
                            stop=(fb == H - 1),
                        )
                    o_stage = opool.tile([128, 512], FP32, tag="ostg", bufs=2)
                    nc.scalar.copy(out=o_stage, in_=ps_o)
                    nc.sync.dma_start(
                        out=outT[ob * 128:(ob + 1) * 128, :], in_=o_stage
                    )

    return nc


def _split_multi_waits(nc, mybir, max_waits=1):
    """This walrus build encodes at most one sync-wait command per
    instruction; Tile attaches one wait per producing proc. Move extra waits
    onto same-engine NoOps inserted immediately before the instruction."""
    n_split = 0
    for f in nc.m.functions:
        for blk in f.blocks:
            ins_list = blk.instructions
            i = 0
            while i < len(ins_list):
                inst = ins_list[i]
                si = getattr(inst, "sync_info", None)
                waits = list(si.on_wait) if si is not None and si.on_wait else []
                if len(waits) > max_waits:
                    si.on_wait = waits[:max_waits]
                    rest = waits[max_waits:]
                    for k in range(0, len(rest), max_waits):
                        nop = mybir.InstNoOp(
                            name=f"{inst.name}_sw{k}",
                            engine=inst.engine,
                            sync_info=mybir.SyncInfo(
                                on_wait=rest[k : k + max_waits], on_update=[]
                            ),
                            bass_nofuse=True,
                        )
                        ins_list.insert(i, nop)
                        i += 1
                    n_split += 1
                i += 1
    return n_split


def _prep_shared(wq, wk, wv, wo):
    """Host-side weight prep: head-feature permutation + tile-major layouts."""
    # permutation: within each head, feature 2f -> f (re), 2f+1 -> 64+f (im)
    perm = np.empty(D, dtype=np.int64)
    for h in range(H):
        base = h * HD
        perm[base:base + 64] = base + 2 * np.arange(64)
        perm[base + 64:base + 128] = base + 2 * np.arange(64) + 1

    def tiles_128(wt):  # wt: [d, o] -> [o_blk, p, d_blk, 128]
        return np.ascontiguousarray(
            wt.reshape(NDB, 128, 16, 128).transpose(2, 1, 0, 3)
        )

    wq_t = tiles_128(wq[perm].T.astype(bf16))
    wk_t = tiles_128(wk[perm].T.astype(bf16))
    wo_t = tiles_128(wo.T.astype(bf16))
    wv_t = np.ascontiguousarray(
        wv.T.astype(bf16).reshape(NDB, 128, 4, 512).transpose(2, 1, 0, 3)
    )

    p = np.zeros((128, 128), dtype=bf16)
    p[np.arange(64) + 64, np.arange(64)] = 1.0
    p[np.arange(64), np.arange(64) + 64] = 1.0
    return wq_t, wk_t, wv_t, wo_t, p


def _prep_core(c, x, freqs_cos, freqs_sin):
    b, j = divmod(c, 4)
    s0 = j * SC

    xpad = np.zeros((KV, D), dtype=np.float32)
    lo = s0 - W
    src_lo = max(lo, 0)
    xpad[src_lo - lo:, :] = x[b, src_lo:s0 + SC, :]
    xT_c = np.ascontiguousarray(
        xpad.T.astype(bf16).reshape(NDB, 128, KV).transpose(1, 0, 2)
    )

    g = np.clip(np.arange(lo, s0 + SC), 0, S - 1)
    cos_g = freqs_cos[g].T.astype(bf16)          # [64, KV]
    sin_g = freqs_sin[g].T
    tabc_c = np.ascontiguousarray(np.concatenate([cos_g, cos_g], axis=0))
    tabs_c = np.ascontiguousarray(
        np.concatenate([-sin_g, sin_g], axis=0).astype(bf16)
    )

    kj = np.arange(128)[:, None, None]
    tb = np.arange(12)[None, :, None]
    qi = np.arange(128)[None, None, :]
    t, blk = tb // 3, tb % 3
    gq = s0 + 128 * t + qi
    gk = s0 - W + 128 * (t + blk) + kj
    valid = (gk >= 0) & (gk <= gq) & (gk > gq - W)
    masks_c = np.ascontiguousarray(valid.astype(bf16))

    return {"xT": xT_c, "tabc": tabc_c, "tabsn": tabs_c, "masks": masks_c}


def kernel(x, freqs_cos, freqs_sin, wq, wk, wv, wo):
    from concourse.bass_utils import run_bass_kernel_spmd

    x = np.asarray(x, dtype=np.float32)
    freqs_cos = np.asarray(freqs_cos, dtype=np.float32)
    freqs_sin = np.asarray(freqs_sin, dtype=np.float32)
    wq = np.asarray(wq, dtype=np.float32)
    wk = np.asarray(wk, dtype=np.float32)
    wv = np.asarray(wv, dtype=np.float32)
    wo = np.asarray(wo, dtype=np.float32)

    if "nc" not in _CACHE:
        import concourse.mybir as mybir

        nc = _build_program()
        _split_multi_waits(nc, mybir)
        _CACHE["nc"] = nc
    nc = _CACHE["nc"]

    wq_t, wk_t, wv_t, wo_t, p = _prep_shared(wq, wk, wv, wo)
    shared = {"wqt": wq_t, "wkt": wk_t, "wvt": wv_t, "wot": wo_t, "p128": p}
    in_maps = []
    for c in range(NCORES):
        m = _prep_core(c, x, freqs_cos, freqs_sin)
        m.update(shared)
        in_maps.append(m)

    res = run_bass_kernel_spmd(nc, in_maps, list(range(NCORES)))

    out = np.empty((B, S, D), dtype=np.float32)
    for c in range(NCORES):
        b, j = divmod(c, 4)
        out[b, j * SC:(j + 1) * SC, :] = res.results[c]["outT"].T
    return out
